# revision 32
# baseline (speedup 1.0000x reference)
"""CBAM kernel for Trainium2 (Bass/Tile), data-parallel over batch on 8 NeuronCores.

Reference computation (per sample):
  ch_att = sigmoid(MLP(mean_hw(x)) + MLP(max_hw(x)))          # [C]
  y      = x * ch_att[:, None, None]
  sp     = conv7x7(concat([mean_c(y), max_c(y)]))             # [H, W]
  out    = y * sigmoid(sp)[None]

Mapping (per core, 2 samples of x[256,128,128] fp32):
  layout: x as [128 partitions (channel%128), 2 blocks, HW] tiles.
  - channel max:  DVE tensor_reduce (2x mode)
  - channel mean: ACT activation(Copy) accum_out
  - MLP: PE matmuls (fp32, exact), ACT relu/sigmoid
  - weighted channel sum (mean_c(y)):  PE matmuls lhsT=ch_att [128,1] (fp32r)
  - channel max of y: DVE tensor_scalar + gpsimd scalar_tensor_tensor ->
      m2 = max(x0*ch0, x1*ch1), then gpsimd partition_all_reduce(max)
  - conv 7x7: 14 PE matmuls with host-built banded matrices (dy banded into
      lhsT, dx as shifted rhs columns, 1/C folded into the avg-channel taps)
  - final: PE replicates sigmoid(conv) row-chunks into PSUM (ones x sp, fp32r),
      DVE scalar_tensor_tensor out = (x * ch) * sp in place, DMA store.
"""

import numpy as np

B = 16          # full batch
N_CORES = 8
B_LOC = B // N_CORES   # 2 samples per core
C = 256
H = W = 128
HW = H * W      # 16384
R = 16
K = 7
NB = 2          # channel blocks of 128
NT = 8          # x tiles per sample
FT = HW // NT   # 2048 free elems per tile
QS = 512        # matmul moving-chunk (fp32 max N)
NQ = FT // QS   # 4

_CACHE = {}


def _build_program():
    import concourse.bass as bass
    import concourse.bacc as bacc
    import concourse.tile as tile
    from concourse import mybir, bass_isa, library_config
    from contextlib import ExitStack

    f32 = mybir.dt.float32
    f16 = mybir.dt.float16
    f32r = mybir.dt.float32r
    AF = mybir.ActivationFunctionType
    ALU = mybir.AluOpType
    AX = mybir.AxisListType

    nc = bacc.Bacc("TRN2", target_bir_lowering=False, debug=False)

    x_ext = nc.declare_dram_parameter("x", [B_LOC, C, H, W], f32r, isOutput=False)
    w1t_ext = nc.declare_dram_parameter("w1t", [128, NB, R], f32, isOutput=False)
    w2t_ext = nc.declare_dram_parameter("w2t", [R, C], f32, isOutput=False)
    wb_ext = nc.declare_dram_parameter("wb", [128, 2 * K, 128], f32, isOutput=False)
    ones_ext = nc.declare_dram_parameter("ones1", [33, 128], f32r, isOutput=False)
    out_ext = nc.declare_dram_parameter("out", [B_LOC, C, H, W], f32r, isOutput=True)

    xr = x_ext[:, :, :, :].rearrange("b (g c) h w -> b c g (h w)", g=NB)
    outr = out_ext[:, :, :, :].rearrange("b (g c) h w -> b c g (h w)", g=NB)

    with tile.TileContext(nc) as tc, ExitStack() as ctx:
        consts = ctx.enter_context(tc.tile_pool(name="consts", bufs=1))
        xpool = ctx.enter_context(tc.tile_pool(name="xp", bufs=NT))
        scratch = ctx.enter_context(tc.tile_pool(name="scr", bufs=3))
        smalls = ctx.enter_context(tc.tile_pool(name="sml", bufs=2))
        spat = ctx.enter_context(tc.tile_pool(name="spat", bufs=1))
        spfp = ctx.enter_context(tc.tile_pool(name="spfp", bufs=1))
        ssfp = ctx.enter_context(tc.tile_pool(name="ssfp", bufs=4))
        pmlp = ctx.enter_context(
            tc.tile_pool(name="pmlp", bufs=1, space=bass.MemorySpace.PSUM))
        pconv = ctx.enter_context(
            tc.tile_pool(name="pconv", bufs=1, space=bass.MemorySpace.PSUM))
        psp = ctx.enter_context(
            tc.tile_pool(name="psp", bufs=2, space=bass.MemorySpace.PSUM))
        prep = ctx.enter_context(
            tc.tile_pool(name="prep", bufs=3, space=bass.MemorySpace.PSUM))

        nc.gpsimd.load_library(library_config.mlp)
        w1t = consts.tile([128, NB, R], f32, tag="w1t")
        nc.sync.dma_start(w1t[:, :, :], w1t_ext[:, :, :])
        w2t = consts.tile([R, C], f32, tag="w2t")
        nc.sync.dma_start(w2t[:, :], w2t_ext[:, :])
        wb = consts.tile([128, 2 * K, 128], f32, tag="wb")
        nc.sync.dma_start(wb[:, :, :], wb_ext[:, :, :])
        ones1 = consts.tile([33, 128], f32r, tag="ones1")
        nc.sync.dma_start(ones1[:, :], ones_ext[:, :])

        for s in range(B_LOC):
            # ---------------- phase A: load + channel stats -------------
            xts = []
            sums = smalls.tile([128, NB, NT], f32, tag="sums")
            maxs = smalls.tile([128, NB, NT], f32, tag="maxs")
            for t in range(NT):
                xt = xpool.tile([128, NB, FT], f32r, tag="xt")
                xts.append(xt)
                nc.sync.dma_start(xt[:, :, :], xr[s, :, :, t * FT:(t + 1) * FT])
                for b in range(NB):
                    dump = scratch.tile([128, FT], f32, tag="scr")
                    nc.scalar.activation(
                        dump[:, :], xt[:, b, :], AF.Copy, bias=0.0, scale=1.0,
                        accum_out=sums[:, b, t:t + 1])
                    nc.vector.tensor_reduce(
                        maxs[:, b, t:t + 1], xt[:, b, :], axis=AX.X, op=ALU.max)

            # ---------------- MLP -> ch_att -----------------------------
            stats = smalls.tile([128, NB, 2], f32, tag="stats")
            for b in range(NB):
                nc.vector.tensor_reduce(
                    stats[:, b, 0:1], sums[:, b, :], axis=AX.X, op=ALU.add)
                nc.vector.tensor_scalar_mul(
                    stats[:, b, 0:1], stats[:, b, 0:1], 1.0 / HW)
                nc.vector.tensor_reduce(
                    stats[:, b, 1:2], maxs[:, b, :], axis=AX.X, op=ALU.max)
            ph = pmlp.tile([R, 2], f32, tag="ph")
            nc.tensor.matmul(ph[:, :], w1t[:, 0, :], stats[:, 0, :],
                             start=True, stop=False)
            nc.tensor.matmul(ph[:, :], w1t[:, 1, :], stats[:, 1, :],
                             start=False, stop=True)
            hmlp = smalls.tile([R, 2], f32, tag="hmlp")
            nc.scalar.activation(hmlp[:, :], ph[:, :], AF.Relu)
            ch = smalls.tile([128, NB], f32, tag="ch")
            for b in range(NB):
                p2 = pmlp.tile([128, 2], f32, tag="p2")
                nc.tensor.matmul(p2[:, :], w2t[:, b * 128:(b + 1) * 128],
                                 hmlp[:, :], start=True, stop=True)
                tsum = smalls.tile([128, 1], f32, tag="tsum")
                nc.vector.tensor_reduce(
                    tsum[:, :], p2[:, :], axis=AX.X, op=ALU.add)
                nc.scalar.activation(ch[:, b:b + 1], tsum[:, :], AF.Sigmoid)

            chr_ = smalls.tile([128, NB], f32r, tag="chr")
            nc.vector.tensor_copy(chr_[:, :], ch[:, :])

            # -------- phase B: spatial stats (sum via PE, max via gpsimd)
            A0 = spat.tile([128, 128], f32, tag="A0")
            A1 = spat.tile([128, 128], f32, tag="A1")

            for t in range(NT):
                xt = xts[t]
                tmp = scratch.tile([128, FT], f32, tag="scr")
                nc.scalar.activation(tmp[:, :], xt[:, 0, :], AF.Copy,
                                     bias=0.0, scale=ch[:, 0:1])
                m2t = scratch.tile([128, FT], f32, tag="scr")
                nc.vector.scalar_tensor_tensor(
                    m2t[:, :], xt[:, 1, :], ch[:, 1:2], tmp[:, :],
                    op0=ALU.mult, op1=ALU.max)
                art = scratch.tile([128, FT], f32, tag="scr")
                nc.gpsimd.partition_all_reduce(
                    art[:, :], m2t[:, :], channels=128,
                    reduce_op=bass_isa.ReduceOp.max)
                # row 0 of art = channel-max for hw in [t*FT, (t+1)*FT)
                nc.sync.dma_start(A1[16 * t:16 * (t + 1), :], art[0:1, :])
                for q in range(NQ):
                    ps = psp.tile([1, QS], f32, tag="ps")
                    sl = slice(q * QS, (q + 1) * QS)
                    nc.tensor.matmul(ps[:, :], chr_[:, 0:1], xt[:, 0, sl],
                                     start=True, stop=False)
                    nc.tensor.matmul(ps[:, :], chr_[:, 1:2], xt[:, 1, sl],
                                     start=False, stop=True)
                    sse = ssfp.tile([1, QS], f32, tag="sse")
                    g = t * NQ + q
                    if g % 2 == 0:
                        nc.scalar.activation(sse[:, :], ps[:, :], AF.Copy)
                    else:
                        nc.vector.tensor_copy(sse[:, :], ps[:, :])
                    nc.sync.dma_start(A0[4 * g:4 * (g + 1), :], sse[:, :])

            # ---------------- conv 7x7 via banded matmuls ---------------
            pc = pconv.tile([128, 128], f32, tag="pc")
            taps = [(1, 3)] + [(c, dx) for c in (1, 0) for dx in range(K)
                               if not (c == 1 and dx == 3)]
            for i, (c, dx) in enumerate(taps):
                sh = dx - 3
                dlo, dhi = max(0, -sh), 128 - max(0, sh)
                A = A0 if c == 0 else A1
                nc.tensor.matmul(
                    pc[:, dlo:dhi], wb[:, c * K + dx, :],
                    A[:, dlo + sh:dhi + sh],
                    start=(i == 0), stop=(i == len(taps) - 1))
            spa = spat.tile([128, 128], f32r, tag="spa")
            nc.scalar.activation(spa[:, :], pc[:, :], AF.Sigmoid)
            # sp flattened onto partition rows 0/32 (matmul base-partition rule)
            spf = spfp.tile([33, 8192], f32r, tag="spf")
            for r in range(2):
                nc.sync.dma_start(spf[32 * r:32 * r + 1, :],
                                  spa[64 * r:64 * (r + 1), :])

            # ---------------- final: out = (x*ch)*sp, store -------------
            for t in range(NT):
                xt = xts[t]
                for q in range(NQ):
                    pr = prep.tile([128, QS], f32, tag="pr")
                    g = t * NQ + q
                    r, off = g // 16, (g % 16) * QS
                    nc.tensor.matmul(
                        pr[:, :],
                        ones1[32 * r:32 * r + 1, :],
                        spf[32 * r:32 * r + 1, off:off + QS],
                        start=True, stop=True)
                    qsl = slice(q * QS, (q + 1) * QS)
                    for b in range(NB):
                        nc.vector.scalar_tensor_tensor(
                            xt[:, b, qsl], xt[:, b, qsl], ch[:, b:b + 1],
                            pr[:, :], op0=ALU.mult, op1=ALU.mult)
                nc.sync.dma_start(outr[s, :, :, t * FT:(t + 1) * FT],
                                  xt[:, :, :])

    nc.compile()
    return nc


def get_program():
    if "nc" not in _CACHE:
        _CACHE["nc"] = _build_program()
    return _CACHE["nc"]


def _host_prep(w1, w2, wconv):
    w1 = np.asarray(w1, dtype=np.float32)
    w2 = np.asarray(w2, dtype=np.float32)
    wconv = np.asarray(wconv, dtype=np.float32)
    # w1t[p, b, j] = w1[j, b*128 + p]
    w1t = np.ascontiguousarray(w1.T.reshape(NB, 128, R).transpose(1, 0, 2))
    w2t = np.ascontiguousarray(w2.T)  # [R, C]
    # banded conv matrices: wb[hp, c*K+dx, h] = keff[c, hp-h+3, dx]
    keff = wconv[0].copy()          # [2, K, K] (dy, dx)
    keff[0] /= C                    # fold the channel-mean divide
    hp = np.arange(128)[:, None]    # h'
    hh = np.arange(128)[None, :]    # h
    dy = hp - hh + 3                # [128, 128]
    valid = (dy >= 0) & (dy < K)
    dyc = np.clip(dy, 0, K - 1)
    wb = np.zeros((128, 2 * K, 128), dtype=np.float32)
    for c in range(2):
        for dx in range(K):
            wb[:, c * K + dx, :] = np.where(valid, keff[c][dyc, dx], 0.0)
    ones1 = np.zeros((33, 128), dtype=np.float32)
    ones1[[0, 32], :] = 1.0
    return w1t, w2t, wb, ones1


def kernel(x, w1, w2, wconv):
    from concourse.bass_utils import run_bass_kernel_spmd

    x = np.ascontiguousarray(np.asarray(x, dtype=np.float32))
    assert x.shape == (B, C, H, W), x.shape
    w1t, w2t, wb, ones1 = _host_prep(w1, w2, wconv)

    nc = get_program()
    in_maps = []
    for i in range(N_CORES):
        in_maps.append({
            "x": x[i * B_LOC:(i + 1) * B_LOC],
            "w1t": w1t, "w2t": w2t, "wb": wb, "ones1": ones1,
        })
    res = run_bass_kernel_spmd(nc, in_maps, list(range(N_CORES)))
    out = np.concatenate([res.results[i]["out"] for i in range(N_CORES)], axis=0)
    return out.astype(np.float32)


# revision 35
# speedup vs baseline: 140.5453x; 140.5453x over previous
"""CBAM kernel for Trainium2 (Bass/Tile), data-parallel over batch on 8 NeuronCores.

Reference computation (per sample):
  ch_att = sigmoid(MLP(mean_hw(x)) + MLP(max_hw(x)))          # [C]
  y      = x * ch_att[:, None, None]
  sp     = conv7x7(concat([mean_c(y), max_c(y)]))             # [H, W]
  out    = y * sigmoid(sp)[None]

Mapping (per core, 2 samples of x[256,128,128] fp32):
  layout: x as [128 partitions (channel%128), 2 blocks, HW] tiles.
  - channel max:  DVE tensor_reduce (2x mode)
  - channel mean: ACT activation(Copy) accum_out
  - MLP: PE matmuls (fp32, exact), ACT relu/sigmoid
  - weighted channel sum (mean_c(y)):  PE matmuls lhsT=ch_att [128,1] (fp32r)
  - channel max of y: DVE tensor_scalar + gpsimd scalar_tensor_tensor ->
      m2 = max(x0*ch0, x1*ch1), then gpsimd partition_all_reduce(max)
  - conv 7x7: 14 PE matmuls with host-built banded matrices (dy banded into
      lhsT, dx as shifted rhs columns, 1/C folded into the avg-channel taps)
  - final: PE replicates sigmoid(conv) row-chunks into PSUM (ones x sp, fp32r),
      DVE scalar_tensor_tensor out = (x * ch) * sp in place, DMA store.
"""

import numpy as np

B = 16          # full batch
N_CORES = 8
B_LOC = B // N_CORES   # 2 samples per core
C = 256
H = W = 128
HW = H * W      # 16384
R = 16
K = 7
NB = 2          # channel blocks of 128
NT = 8          # x tiles per sample
FT = HW // NT   # 2048 free elems per tile
FB = 1024       # phase-B sub-chunk
RPB = FB // W   # 8 A1 rows per sub-chunk
QS = 512        # matmul moving-chunk (fp32 max N)
NQ = FT // QS   # 4

_CACHE = {}


def _build_program(repeat=1):
    import concourse.bass as bass
    import concourse.bacc as bacc
    import concourse.tile as tile
    from concourse import mybir, bass_isa, library_config
    from contextlib import ExitStack

    f32 = mybir.dt.float32
    f16 = mybir.dt.float16
    f32r = mybir.dt.float32r
    AF = mybir.ActivationFunctionType
    ALU = mybir.AluOpType
    AX = mybir.AxisListType

    nc = bacc.Bacc("TRN2", target_bir_lowering=False, debug=False)

    x_ext = nc.declare_dram_parameter("x", [B_LOC, C, H, W], f32r, isOutput=False)
    w1t_ext = nc.declare_dram_parameter("w1t", [128, NB, R], f32, isOutput=False)
    w2t_ext = nc.declare_dram_parameter("w2t", [R, C], f32, isOutput=False)
    wb_ext = nc.declare_dram_parameter("wb", [128, 2 * K, 128], f32, isOutput=False)
    ones_ext = nc.declare_dram_parameter("ones1", [33, 128], f32r, isOutput=False)
    out_ext = nc.declare_dram_parameter("out", [B_LOC, C, H, W], f32r, isOutput=True)

    xr = x_ext[:, :, :, :].rearrange("b (g c) h w -> b c g (h w)", g=NB)
    outr = out_ext[:, :, :, :].rearrange("b (g c) h w -> b c g (h w)", g=NB)

    with tile.TileContext(nc) as tc, ExitStack() as ctx:
        consts = ctx.enter_context(tc.tile_pool(name="consts", bufs=1))
        xpool = ctx.enter_context(tc.tile_pool(name="xp", bufs=NT))
        scratch = ctx.enter_context(tc.tile_pool(name="scr", bufs=6))
        smalls = ctx.enter_context(tc.tile_pool(name="sml", bufs=2))
        spat = ctx.enter_context(tc.tile_pool(name="spat", bufs=1))
        spfp = ctx.enter_context(tc.tile_pool(name="spfp", bufs=1))
        ssfp = ctx.enter_context(tc.tile_pool(name="ssfp", bufs=4))
        pmlp = ctx.enter_context(
            tc.tile_pool(name="pmlp", bufs=1, space=bass.MemorySpace.PSUM))
        pconv = ctx.enter_context(
            tc.tile_pool(name="pconv", bufs=1, space=bass.MemorySpace.PSUM))
        psp = ctx.enter_context(
            tc.tile_pool(name="psp", bufs=2, space=bass.MemorySpace.PSUM))
        prep = ctx.enter_context(
            tc.tile_pool(name="prep", bufs=3, space=bass.MemorySpace.PSUM))

        nc.gpsimd.load_library(library_config.mlp)
        w1t = consts.tile([128, NB, R], f32, tag="w1t")
        nc.sync.dma_start(w1t[:, :, :], w1t_ext[:, :, :])
        w2t = consts.tile([R, C], f32, tag="w2t")
        nc.sync.dma_start(w2t[:, :], w2t_ext[:, :])
        wb = consts.tile([128, 2 * K, 128], f32, tag="wb")
        nc.sync.dma_start(wb[:, :, :], wb_ext[:, :, :])
        ones1 = consts.tile([33, 128], f32r, tag="ones1")
        nc.sync.dma_start(ones1[:, :], ones_ext[:, :])

        for s in [s_ for _ in range(repeat) for s_ in range(B_LOC)]:
            # ---------------- phase A: load + channel stats -------------
            xts = []
            sums = smalls.tile([128, NB, NT], f32, tag="sums")
            maxs = smalls.tile([128, NB, NT], f32, tag="maxs")
            for t in range(NT):
                xt = xpool.tile([128, NB, FT], f32r, tag="xt")
                xts.append(xt)
                nc.sync.dma_start(xt[:, :, :], xr[s, :, :, t * FT:(t + 1) * FT])
                for b in range(NB):
                    dump = scratch.tile([128, FT], f32, tag="scr")
                    nc.scalar.activation(
                        dump[:, :], xt[:, b, :], AF.Copy, bias=0.0, scale=1.0,
                        accum_out=sums[:, b, t:t + 1])
                    nc.vector.tensor_reduce(
                        maxs[:, b, t:t + 1], xt[:, b, :], axis=AX.X, op=ALU.max)

            # ---------------- MLP -> ch_att -----------------------------
            stats = smalls.tile([128, NB, 2], f32, tag="stats")
            for b in range(NB):
                nc.vector.tensor_reduce(
                    stats[:, b, 0:1], sums[:, b, :], axis=AX.X, op=ALU.add)
                nc.vector.tensor_scalar_mul(
                    stats[:, b, 0:1], stats[:, b, 0:1], 1.0 / HW)
                nc.vector.tensor_reduce(
                    stats[:, b, 1:2], maxs[:, b, :], axis=AX.X, op=ALU.max)
            ph = pmlp.tile([R, 2], f32, tag="ph")
            nc.tensor.matmul(ph[:, :], w1t[:, 0, :], stats[:, 0, :],
                             start=True, stop=False)
            nc.tensor.matmul(ph[:, :], w1t[:, 1, :], stats[:, 1, :],
                             start=False, stop=True)
            hmlp = smalls.tile([R, 2], f32, tag="hmlp")
            nc.scalar.activation(hmlp[:, :], ph[:, :], AF.Relu)
            ch = smalls.tile([128, NB], f32, tag="ch")
            for b in range(NB):
                p2 = pmlp.tile([128, 2], f32, tag="p2")
                nc.tensor.matmul(p2[:, :], w2t[:, b * 128:(b + 1) * 128],
                                 hmlp[:, :], start=True, stop=True)
                tsum = smalls.tile([128, 1], f32, tag="tsum")
                nc.vector.tensor_reduce(
                    tsum[:, :], p2[:, :], axis=AX.X, op=ALU.add)
                nc.scalar.activation(ch[:, b:b + 1], tsum[:, :], AF.Sigmoid)

            chr_ = smalls.tile([128, NB], f32r, tag="chr")
            nc.vector.tensor_copy(chr_[:, :], ch[:, :])

            # -------- phase B: spatial stats (sum via PE, max via gpsimd)
            A0 = spat.tile([128, 128], f32, tag="A0")
            A1 = spat.tile([128, 128], f32, tag="A1")

            for t in range(NT):
                xt = xts[t]
                tmp = scratch.tile([128, FT], f32, tag="scr")
                nc.scalar.activation(tmp[:, :], xt[:, 0, :], AF.Copy,
                                     bias=0.0, scale=ch[:, 0:1])
                m2t = scratch.tile([128, FT], f32, tag="scr")
                nc.vector.scalar_tensor_tensor(
                    m2t[:, :], xt[:, 1, :], ch[:, 1:2], tmp[:, :],
                    op0=ALU.mult, op1=ALU.max)
                art = scratch.tile([128, FT], f32, tag="scr")
                nc.gpsimd.partition_all_reduce(
                    art[:, :], m2t[:, :], channels=128,
                    reduce_op=bass_isa.ReduceOp.max)
                # row 0 of art = channel-max for hw in [t*FT, (t+1)*FT)
                nc.sync.dma_start(A1[RPB * u:RPB * (u + 1), :], art[0:1, :])
                for q in range(FB // QS):
                    ps = psp.tile([1, QS], f32, tag="ps")
                    sl = slice(bo + q * QS, bo + (q + 1) * QS)
                    nc.tensor.matmul(ps[:, :], chr_[:, 0:1], xt[:, 0, sl],
                                     start=True, stop=False)
                    nc.tensor.matmul(ps[:, :], chr_[:, 1:2], xt[:, 1, sl],
                                     start=False, stop=True)
                    sse = ssfp.tile([1, QS], f32, tag="sse")
                    g = (u * FB) // QS + q
                    if g % 2 == 0:
                        nc.scalar.activation(sse[:, :], ps[:, :], AF.Copy)
                    else:
                        nc.vector.tensor_copy(sse[:, :], ps[:, :])
                    nc.sync.dma_start(A0[4 * g:4 * (g + 1), :], sse[:, :])

            # ---------------- conv 7x7 via banded matmuls ---------------
            pc = pconv.tile([128, 128], f32, tag="pc")
            taps = [(1, 3)] + [(c, dx) for c in (1, 0) for dx in range(K)
                               if not (c == 1 and dx == 3)]
            for i, (c, dx) in enumerate(taps):
                sh = dx - 3
                dlo, dhi = max(0, -sh), 128 - max(0, sh)
                A = A0 if c == 0 else A1
                nc.tensor.matmul(
                    pc[:, dlo:dhi], wb[:, c * K + dx, :],
                    A[:, dlo + sh:dhi + sh],
                    start=(i == 0), stop=(i == len(taps) - 1))
            spa = spat.tile([128, 128], f32r, tag="spa")
            nc.scalar.activation(spa[:, :], pc[:, :], AF.Sigmoid)
            # sp flattened onto partition rows 0/32 (matmul base-partition rule)
            spf = spfp.tile([33, 8192], f32r, tag="spf")
            for r in range(2):
                nc.sync.dma_start(spf[32 * r:32 * r + 1, :],
                                  spa[64 * r:64 * (r + 1), :])

            # ---------------- final: out = (x*ch)*sp, store -------------
            for t in range(NT):
                xt = xts[t]
                for q in range(NQ):
                    pr = prep.tile([128, QS], f32, tag="pr")
                    g = t * NQ + q
                    r, off = g // 16, (g % 16) * QS
                    nc.tensor.matmul(
                        pr[:, :],
                        ones1[32 * r:32 * r + 1, :],
                        spf[32 * r:32 * r + 1, off:off + QS],
                        start=True, stop=True)
                    qsl = slice(q * QS, (q + 1) * QS)
                    for b in range(NB):
                        nc.vector.scalar_tensor_tensor(
                            xt[:, b, qsl], xt[:, b, qsl], ch[:, b:b + 1],
                            pr[:, :], op0=ALU.mult, op1=ALU.mult)
                nc.sync.dma_start(outr[s, :, :, t * FT:(t + 1) * FT],
                                  xt[:, :, :])

    nc.compile()
    return nc


def get_program(repeat=1):
    key = ("nc", repeat)
    if key not in _CACHE:
        _CACHE[key] = _build_program(repeat)
    return _CACHE[key]


def _host_prep(w1, w2, wconv):
    w1 = np.asarray(w1, dtype=np.float32)
    w2 = np.asarray(w2, dtype=np.float32)
    wconv = np.asarray(wconv, dtype=np.float32)
    # w1t[p, b, j] = w1[j, b*128 + p]
    w1t = np.ascontiguousarray(w1.T.reshape(NB, 128, R).transpose(1, 0, 2))
    w2t = np.ascontiguousarray(w2.T)  # [R, C]
    # banded conv matrices: wb[hp, c*K+dx, h] = keff[c, hp-h+3, dx]
    keff = wconv[0].copy()          # [2, K, K] (dy, dx)
    keff[0] /= C                    # fold the channel-mean divide
    hp = np.arange(128)[:, None]    # h'
    hh = np.arange(128)[None, :]    # h
    dy = hp - hh + 3                # [128, 128]
    valid = (dy >= 0) & (dy < K)
    dyc = np.clip(dy, 0, K - 1)
    wb = np.zeros((128, 2 * K, 128), dtype=np.float32)
    for c in range(2):
        for dx in range(K):
            wb[:, c * K + dx, :] = np.where(valid, keff[c][dyc, dx], 0.0)
    ones1 = np.zeros((33, 128), dtype=np.float32)
    ones1[[0, 32], :] = 1.0
    return w1t, w2t, wb, ones1


def kernel(x, w1, w2, wconv):
    from concourse.bass_utils import run_bass_kernel_spmd

    x = np.ascontiguousarray(np.asarray(x, dtype=np.float32))
    assert x.shape == (B, C, H, W), x.shape
    w1t, w2t, wb, ones1 = _host_prep(w1, w2, wconv)

    nc = get_program()
    in_maps = []
    for i in range(N_CORES):
        in_maps.append({
            "x": x[i * B_LOC:(i + 1) * B_LOC],
            "w1t": w1t, "w2t": w2t, "wb": wb, "ones1": ones1,
        })
    res = run_bass_kernel_spmd(nc, in_maps, list(range(N_CORES)))
    out = np.concatenate([res.results[i]["out"] for i in range(N_CORES)], axis=0)
    return out.astype(np.float32)
